# revision 1
# baseline (speedup 1.0000x reference)
"""DSGCN block kernel for 8x Trainium2 NeuronCores (Bass/Tile).

Reference (B=16, T=128, N=64, C=128, O=256, K=3, kt=3):
  s[k,n] = sum_m A[k,n,m]          (the einsum contracts A over m only)
  h[b,t,n,o]   = sum_c x[b,t,n,c] * W_eff[n][c,o],
                 W_eff[n] = sum_k s[k,n] * diag(dw[k]) @ W_pw[:, kC:(k+1)C].T
  h_conv       = depthwise conv over t (taps W_conv[o,0,j], pad 1)
  h_gn         = GroupNorm(8 groups over (32 o-chans x all t)) per (b,n)
  y            = h_gn + x @ W_res.T
  out          = gelu(LayerNorm_o(y))    (exact erf gelu; ln_w=1, ln_b=0)

Device strategy (per core, nodes n in [8i, 8i+8)):
  - conv folded into PE: psum_h[o,t] = sum_j MM(W_eff[n]*tap_j, xT shifted)
  - sample tile = (b, n) -> [t=128, c=128]; batches of 4 samples, same n
  - fp16 on-chip compute, fp32 PSUM/stats
  - A-phase (per batch): PE transpose x->xT; conv MMs -> psum_h;
    ACT fat drain psum_h -> h_raw fp16; DVE bn_stats (multi-window)
  - per n-group: GroupNorm finalize (fold even/odd stats, group fold via
    tiny matmuls with G4/(1/32), rsqrt via magic+Newton on DVE) -> alpha,beta
  - B-phase (per batch): DVE apply (h*alpha+beta) -> ypw; PE builds
    psum_fin[t, 258] = x@[W_res.T|omega|0] + ypw^T (+ ones-cols = LN sums);
    ACT drain -> yT; DVE LN sumsq/finalize + (y-mu)*inv; ACT fat gelu
  - SWDGE cast DMAs: loads fp32->fp16, stores fp16->fp32
"""

import numpy as np

import concourse.bass as bass
import concourse.tile as tile
from concourse import mybir
from concourse.tile import TileContext, ScopedClock
from concourse.bass_utils import run_bass_kernel_spmd

B, T, N, C, O, K, KT = 16, 128, 64, 128, 256, 3, 3
EPS = 1e-5
NUM_GROUPS = 8
GSIZE = O // NUM_GROUPS
NCORES = 8
NLOC = N // NCORES            # 8 nodes per core
BATCH = 4                     # samples per batch (same n)
NB = B // BATCH               # 4 batches per n-group
FP16 = mybir.dt.float16
FP32 = mybir.dt.float32
AL = mybir.AluOpType

_COMPILED = {}


def _split_excess_waits(nc):
    """This walrus build allows at most ONE semaphore wait per instruction
    (any opcode). Tile emits multi-wait instructions; rewrite each into
    single-wait NOPs (same engine, immediately before) + the instruction
    keeping one wait."""
    wid = [0]
    for fn in nc.m.functions:
        for blk in fn.blocks:
            insts = list(blk.instructions)
            out = []
            changed = False
            for inst in insts:
                si = inst.sync_info
                waits = list(si.on_wait) if si and si.on_wait else []
                if len(waits) > 1:
                    changed = True
                    for w in waits[:-1]:
                        nop = mybir.InstNoOp(name=f"WSPLIT-{wid[0]}", ins=[], outs=[])
                        wid[0] += 1
                        nop.engine = inst.engine
                        nop.sync_info = mybir.SyncInfo(on_wait=[w], on_update=[])
                        out.append(nop)
                    si.on_wait = [waits[-1]]
                out.append(inst)
            if changed:
                blk.instructions = out


def _host_tables(A, dw, W_pw, W_conv, W_res, gn_w, gn_b):
    s = A.sum(axis=2)                                    # [K, N]
    Wk = np.empty((K, C, O), np.float32)
    for k in range(K):
        Wk[k] = dw[k][:, None] * W_pw[:, k * C:(k + 1) * C].T
    W_eff = np.einsum("kn,kco->nco", s, Wk)              # [N, C, O]
    taps = W_conv[:, 0, :]                               # [O, KT]
    W_eff_dt = W_eff[:, None, :, :] * taps.T[None, :, None, :]  # [N,KT,C,O]

    wres = np.ascontiguousarray(W_res.T)

    ident = np.eye(128, dtype=np.float16)

    g4 = np.zeros((128, 4), np.float16)
    g4t = np.zeros((4, 128), np.float16)
    for g in range(4):
        g4[g * 32:(g + 1) * 32, g] = 1.0 / GSIZE
        g4t[g, g * 32:(g + 1) * 32] = 1.0

    gnwb = np.zeros((128, 4), np.float32)                # w_blk0 w_blk1 b_blk0 b_blk1
    gnwb[:, 0] = gn_w[:128]
    gnwb[:, 1] = gn_w[128:]
    gnwb[:, 2] = gn_b[:128]
    gnwb[:, 3] = gn_b[128:]

    return {
        "wdt": W_eff_dt.astype(np.float16),
        "wres": wres.astype(np.float16),
        "ident": ident,
        "g4": g4,
        "g4t": g4t,
        "gnwb": gnwb,
    }


def _rsqrt(nc, pool, out, v, eps, tag, eng=None):
    """out = 1/sqrt(v + eps), magic-number + Newton iterations."""
    if eng is None:
        eng = nc.vector
    shp = list(v.shape)
    ve = pool.tile(shp, FP32, tag=f"{tag}_ve")
    eng.tensor_scalar(ve, v, float(eps), None, AL.add)
    vi = ve.bitcast(mybir.dt.int32)
    y = pool.tile(shp, FP32, tag=f"{tag}_y")
    yi = y.bitcast(mybir.dt.int32)
    # yi = MAGIC - (vi >> 1)  ==  ((vi >> 1) ^ 0xFFFFFFFF) + (MAGIC + 1)
    eng.tensor_scalar(yi, vi, 1, -1, AL.logical_shift_right, AL.bitwise_xor)
    eng.tensor_scalar(yi, yi, 0x5F3759DF + 1, None, AL.add)
    half = pool.tile(shp, FP32, tag=f"{tag}_h")
    eng.tensor_scalar(half, ve, 0.5, None, AL.mult)
    t1 = pool.tile(shp, FP32, tag=f"{tag}_t")
    for _ in range(2):
        eng.tensor_tensor(t1, y, y, AL.mult)
        eng.tensor_tensor(t1, t1, half, AL.mult)
        eng.tensor_scalar(t1, t1, -1.0, 1.5, AL.mult, AL.add)
        eng.tensor_tensor(y, y, t1, AL.mult)
    eng.tensor_copy(out, y)


def _build_kernel():
    nc = bass.Bass("TRN2")

    x_d = nc.dram_tensor("x", [B, T, NLOC, C], FP32, kind="ExternalInput")
    wdt_d = nc.dram_tensor("wdt", [NLOC, KT, C, O], FP16, kind="ExternalInput")
    wres_d = nc.dram_tensor("wres", [C, O], FP16, kind="ExternalInput")
    ident_d = nc.dram_tensor("ident", [128, 128], FP16, kind="ExternalInput")
    g4_d = nc.dram_tensor("g4", [128, 4], FP16, kind="ExternalInput")
    g4t_d = nc.dram_tensor("g4t", [4, 128], FP16, kind="ExternalInput")
    gnwb_d = nc.dram_tensor("gnwb", [128, 4], FP32, kind="ExternalInput")
    out_d = nc.dram_tensor("out", [B, T, NLOC, O], FP32, kind="ExternalOutput")

    with TileContext(nc) as tc:
        with (
            tc.tile_pool(name="const", bufs=1) as cst,
            tc.tile_pool(name="stage", bufs=4) as stage,
            tc.tile_pool(name="work", bufs=3) as work,
            tc.tile_pool(name="bwork", bufs=6) as bwork,
            tc.tile_pool(name="stat", bufs=4) as stat,
            tc.tile_pool(name="tiny", bufs=6) as tiny,
            tc.tile_pool(name="psA", bufs=1, space="PSUM") as psA,
            tc.tile_pool(name="psH", bufs=1, space="PSUM") as psH,
            tc.tile_pool(name="psT", bufs=1, space="PSUM") as psT,
            tc.tile_pool(name="psF", bufs=2, space="PSUM") as psF,
        ):
            ident = cst.tile([128, 128], FP16)
            nc.sync.dma_start(out=ident, in_=ident_d.ap())
            wres = cst.tile([C, O], FP16)
            nc.sync.dma_start(out=wres, in_=wres_d.ap())
            g4 = cst.tile([128, 4], FP16)
            nc.sync.dma_start(out=g4, in_=g4_d.ap())
            g4t = cst.tile([4, 128], FP16)
            nc.sync.dma_start(out=g4t, in_=g4t_d.ap())
            gnwb = cst.tile([128, 4], FP32)
            nc.sync.dma_start(out=gnwb, in_=gnwb_d.ap())

            def emit_A(ni):
                # ---- stage loads for node ni
                xs = stage.tile([T, B, C], FP16, tag="xs")
                nc.gpsimd.dma_start(
                    out=xs, in_=x_d.ap()[:, :, ni, :].transpose([1, 0, 2])
                )
                wdt = stage.tile([C, KT, O], FP16, tag="wdt")
                nc.sync.dma_start(
                    out=wdt, in_=wdt_d.ap()[ni].transpose([1, 0, 2])
                )
                gout = stage.tile([T, B, O], FP16, tag="gout")

                xt = work.tile([C, B * T], FP16, tag="xt")
                h_raw = work.tile([128, 2, B * T], FP16, tag="h_raw")
                stats = stat.tile([128, 2, B, 6], FP32, tag="stats")

                # ================= A phase =================
                for bi in range(NB):
                    s0 = bi * BATCH
                    ps_xt = psA.tile([C, BATCH * T], FP16, tag="ps_xt")
                    for s in range(BATCH):
                        nc.tensor.transpose(
                            ps_xt[:, s * T:(s + 1) * T],
                            xs[:, s0 + s, :],
                            ident,
                        )
                    nc.vector.tensor_copy(xt[:, s0 * T:(s0 + BATCH) * T], ps_xt)

                    ps_h = psH.tile([128, 2, BATCH * T], FP32, tag="ps_h")
                    for blk in range(2):
                        ob = slice(blk * 128, (blk + 1) * 128)
                        nc.tensor.matmul(
                            ps_h[:, blk, :],
                            lhsT=wdt[:, 1, ob],
                            rhs=xt[:, s0 * T:(s0 + BATCH) * T],
                            start=True, stop=False,
                        )
                        for s in range(BATCH):
                            c0 = (s0 + s) * T
                            nc.tensor.matmul(
                                ps_h[:, blk, s * T + 1:(s + 1) * T],
                                lhsT=wdt[:, 0, ob],
                                rhs=xt[:, c0:c0 + T - 1],
                                start=False, stop=False,
                            )
                            nc.tensor.matmul(
                                ps_h[:, blk, s * T:(s + 1) * T - 1],
                                lhsT=wdt[:, 2, ob],
                                rhs=xt[:, c0 + 1:c0 + T],
                                start=False, stop=(s == BATCH - 1),
                            )
                    # fat fp32->fp16 drains (ACT), one per block
                    for blk in range(2):
                        nc.scalar.copy(
                            h_raw[:, blk, s0 * T:(s0 + BATCH) * T],
                            ps_h[:, blk, :],
                        )
                    # bn_stats windows (this walrus requires out == [p, 6])
                    for blk in range(2):
                        for s in range(BATCH):
                            nc.vector.bn_stats(
                                out=stats[:, blk, s0 + s, :],
                                in_=h_raw[:, blk, (s0 + s) * T:(s0 + s + 1) * T],
                            )

                # ============ GroupNorm finalize (per n-group) ============
                # even/odd fold: P1 = 0.5*(me+mo); P2 = (cve+cvo)/128 + 0.5*(me^2+mo^2)
                me = stats[:, :, :, 1]
                mo = stats[:, :, :, 4]
                cve = stats[:, :, :, 2]
                cvo = stats[:, :, :, 5]
                mm = stat.tile([128, 2, B, 2], FP16, tag="mmtile")
                t1 = tiny.tile([128, 2, B], FP32, tag="gn_t1")
                nc.vector.tensor_tensor(t1, me, mo, AL.add)
                nc.vector.tensor_scalar(mm[:, :, :, 0], t1, 0.5, None, AL.mult)
                t2 = tiny.tile([128, 2, B], FP32, tag="gn_t2")
                nc.vector.tensor_tensor(t2, cve, cvo, AL.add)
                t3 = tiny.tile([128, 2, B], FP32, tag="gn_t3")
                nc.vector.tensor_tensor(t3, me, me, AL.mult)
                t4 = tiny.tile([128, 2, B], FP32, tag="gn_t4")
                nc.vector.tensor_tensor(t4, mo, mo, AL.mult)
                nc.vector.tensor_tensor(t3, t3, t4, AL.add)
                nc.vector.tensor_scalar(t2, t2, 1.0 / 128.0, None, AL.mult)
                # mm[...,1] = 0.5*t3 + t2
                nc.vector.scalar_tensor_tensor(
                    mm[:, :, :, 1], t3, 0.5, t2, AL.mult, AL.add
                )
                # group fold: both blocks land on partitions 0..3 (free-major blk)
                ps_g = psT.tile([4, 2, B * 2], FP32, tag="ps_tiny")
                for blk in range(2):
                    nc.tensor.matmul(
                        ps_g[:, blk, :],
                        lhsT=g4,
                        rhs=mm[:, blk].rearrange("p s d -> p (s d)"),
                        start=True, stop=True,
                    )
                gstat = tiny.tile([4, 2, B, 2], FP32, tag="gstat")
                nc.vector.tensor_copy(
                    gstat.rearrange("g b s d -> g (b s d)"),
                    ps_g.rearrange("g b s -> g (b s)"),
                )
                gvar = tiny.tile([4, 2, B], FP32, tag="gvar")
                nc.vector.tensor_tensor(
                    gvar, gstat[:, :, :, 0], gstat[:, :, :, 0], AL.mult
                )
                nc.vector.tensor_tensor(
                    gvar, gstat[:, :, :, 1], gvar, AL.subtract
                )
                grs = tiny.tile([4, 2, B], FP32, tag="grs")
                _rsqrt(nc, tiny, grs, gvar, EPS, "gn")
                # rsmu [4, 2, 2B] fp16 = per blk [rs | mu]
                rsmu = tiny.tile([4, 2, 2 * B], FP16, tag="rsmu")
                nc.vector.tensor_copy(rsmu[:, :, 0:B], grs)
                nc.vector.tensor_copy(rsmu[:, :, B:], gstat[:, :, :, 0])
                # broadcast groups -> 128 partitions per blk
                ps_ab = psT.tile([128, 2, 2 * B], FP32, tag="ps_tiny")
                for blk in range(2):
                    nc.tensor.matmul(
                        ps_ab[:, blk, :],
                        lhsT=g4t[:, 0:128],
                        rhs=rsmu[:, blk, :],
                        start=True, stop=True,
                    )
                ab = stat.tile([128, 2, 2 * B], FP32, tag="ab")
                nc.vector.tensor_copy(ab, ps_ab)
                alpha = stat.tile([128, 2, B], FP32, tag="alpha")
                beta = stat.tile([128, 2, B], FP32, tag="beta")
                for blk in range(2):
                    nc.vector.tensor_scalar(
                        alpha[:, blk], ab[:, blk, 0:B],
                        gnwb[:, blk:blk + 1], None, AL.mult,
                    )
                    nc.vector.tensor_tensor(
                        beta[:, blk], ab[:, blk, B:], alpha[:, blk], AL.mult
                    )
                    nc.vector.tensor_scalar(
                        beta[:, blk], beta[:, blk],
                        -1.0, gnwb[:, 2 + blk:3 + blk], AL.mult, AL.add,
                    )

                return dict(xt=xt, h_raw=h_raw, alpha=alpha, beta=beta,
                            gout=gout, wdt=wdt)

            def emit_B(ni, c):
                xt = c["xt"]; h_raw = c["h_raw"]; alpha = c["alpha"]
                beta = c["beta"]; gout = c["gout"]
                yt_all = work.tile([T, B, O], FP16, tag="yt_all")
                lstats = stat.tile([T, B, 6], FP32, tag="lstats")
                for bi in range(NB):
                    s0 = bi * BATCH
                    ypw = bwork.tile([128, 2, BATCH * T], FP16, tag="ypw")
                    for blk in range(2):
                        for s in range(BATCH):
                            nc.gpsimd.tensor_scalar(
                                ypw[:, blk, s * T:(s + 1) * T],
                                h_raw[:, blk, (s0 + s) * T:(s0 + s + 1) * T],
                                alpha[:, blk, s0 + s:s0 + s + 1],
                                beta[:, blk, s0 + s:s0 + s + 1],
                                AL.mult, AL.add,
                            )
                    for half in range(2):
                        ps_fin = psF.tile([T, 2, 512], FP32, tag="ps_fin")
                        for si in range(2):
                            sl = half * 2 + si
                            s = s0 + sl
                            nc.tensor.matmul(
                                ps_fin[:, si, 0:O],
                                lhsT=xt[:, s * T:(s + 1) * T],
                                rhs=wres,
                                start=True, stop=False,
                            )
                            for blk in range(2):
                                nc.tensor.matmul(
                                    ps_fin[:, si, blk * 128:blk * 128 + 128],
                                    lhsT=ypw[:, blk, sl * T:(sl + 1) * T],
                                    rhs=ident,
                                    start=False, stop=(blk == 1),
                                )
                        nc.scalar.copy(
                            yt_all[:, s0 + half * 2:s0 + half * 2 + 2, :],
                            ps_fin[:, :, 0:O],
                        )
                        for si in range(2):
                            s = s0 + half * 2 + si
                            nc.vector.bn_stats(
                                out=lstats[:, s, :], in_=yt_all[:, s, :]
                            )

                # ---- LayerNorm finalize, whole n-group [T, B]
                lme = lstats[:, :, 1]
                lmo = lstats[:, :, 4]
                lcve = lstats[:, :, 2]
                lcvo = lstats[:, :, 5]
                lmu = stat.tile([T, B], FP32, tag="lmu")
                nc.vector.tensor_tensor(lmu, lme, lmo, AL.add)
                nc.vector.tensor_scalar(lmu, lmu, 0.5, None, AL.mult)
                lt2 = tiny.tile([T, B], FP32, tag="lt2")
                nc.vector.tensor_tensor(lt2, lcve, lcvo, AL.add)
                lt3 = tiny.tile([T, B], FP32, tag="lt3")
                nc.vector.tensor_tensor(lt3, lme, lme, AL.mult)
                lt4 = tiny.tile([T, B], FP32, tag="lt4")
                nc.vector.tensor_tensor(lt4, lmo, lmo, AL.mult)
                nc.vector.tensor_tensor(lt3, lt3, lt4, AL.add)
                nc.vector.tensor_scalar(lt2, lt2, 1.0 / O, None, AL.mult)
                lvar = tiny.tile([T, B], FP32, tag="lvar")
                nc.vector.scalar_tensor_tensor(
                    lvar, lt3, 0.5, lt2, AL.mult, AL.add
                )
                lm2 = tiny.tile([T, B], FP32, tag="lm2")
                nc.vector.tensor_tensor(lm2, lmu, lmu, AL.mult)
                nc.vector.tensor_tensor(lvar, lvar, lm2, AL.subtract)
                linv = stat.tile([T, B], FP32, tag="linv")
                _rsqrt(nc, tiny, linv, lvar, EPS, "ln")

                # ---- gelu with fused LN affine: gelu(linv*y + (-mu*linv))
                lnb = stat.tile([T, B], FP32, tag="lnb")
                nc.vector.tensor_tensor(lnb, lmu, linv, AL.mult)
                nc.vector.tensor_scalar(lnb, lnb, -1.0, None, AL.mult)
                for s in range(B):
                    nc.scalar.activation(
                        out=gout[:, s, :],
                        in_=yt_all[:, s, :],
                        func=mybir.ActivationFunctionType.Gelu,
                        bias=lnb[:, s:s + 1],
                        scale=linv[:, s:s + 1],
                    )

                nc.gpsimd.dma_start(
                    out=out_d.ap()[:, :, ni, :].transpose([1, 0, 2]),
                    in_=gout,
                )

            for ni in range(NLOC):
                emit_B(ni, emit_A(ni))

    _split_excess_waits(nc)
    return nc


LAST_RESULT = {}


def kernel(x, A, dw_weights, W_pw, W_conv, gn_w, gn_b, ln_w, ln_b, W_res,
           _trace=False):
    x = np.asarray(x, np.float32)
    assert np.allclose(np.asarray(ln_w), 1.0) and np.allclose(np.asarray(ln_b), 0.0)
    tabs = _host_tables(
        np.asarray(A, np.float32), np.asarray(dw_weights, np.float32),
        np.asarray(W_pw, np.float32), np.asarray(W_conv, np.float32),
        np.asarray(W_res, np.float32), np.asarray(gn_w, np.float32),
        np.asarray(gn_b, np.float32),
    )
    if "nc" not in _COMPILED:
        _COMPILED["nc"] = _build_kernel()
    nc = _COMPILED["nc"]

    in_maps = []
    for core in range(NCORES):
        nsl = slice(core * NLOC, (core + 1) * NLOC)
        in_maps.append({
            "x": np.ascontiguousarray(x[:, :, nsl, :]),
            "wdt": np.ascontiguousarray(tabs["wdt"][nsl]),
            "wres": tabs["wres"],
            "ident": tabs["ident"],
            "g4": tabs["g4"],
            "g4t": tabs["g4t"],
            "gnwb": tabs["gnwb"],
        })
    kw = {}
    if _trace:
        try:
            import antenv.axon_hooks  # noqa: F401
            kw = dict(trace=True, stitch_traces=False)
        except ImportError:
            pass
    res = run_bass_kernel_spmd(nc, in_maps, core_ids=list(range(NCORES)), **kw)
    LAST_RESULT["res"] = res
    out = np.empty((B, T, N, O), np.float32)
    for core in range(NCORES):
        out[:, :, core * NLOC:(core + 1) * NLOC, :] = res.results[core]["out"]
    return out

